# revision 51
# baseline (speedup 1.0000x reference)
"""TENER multi-head self-attention Trainium2 kernel (8-core batch-parallel).

Math transformation (eliminates the [T,2T] skew/shift tensor entirely):
  rel[i,j] = (q_i + bq + v_bias_h) . pe(j-i),  pe(r)=[sin(r*d_f), cos(r*d_f)]
With angle-difference identities this becomes a plain matmul:
  rel[i,j] = a_i . sin(j*d) + b_i . cos(j*d)
so the full logits are one K=128 contraction of [q ; rot(q')] against
[k ; pe0] per head.  Softmax runs without max-subtraction (max logit ~61
< 88), row sums come from an appended ones-column in the PV matmul.

Matmul inputs are 16-bit: fp16 on the q/k/logits path (absolute logit
error ~5e-3, negligible through exp), bf16 for exp/value (exp reaches
e^61 and needs fp32-range exponents).

Structure: strictly sequential phases (projections, then attention,
then output) — each phase's PSUM pool closes before the next opens, and
within the attention phase a flat (head, key-block) pipeline keeps the
scalar engine's exp chain (the critical path) gapless with logits
emitted two iterations ahead.  Warm-up matmuls run during the initial
DMA window so the PE HAM clock gate reaches 2.4 GHz before real work.

Sharding: data-parallel over batch B=8 -> core c computes batch c.
"""
import math
import sys

sys.path.insert(0, "/opt/trn_rl_repo")

import numpy as np
import ml_dtypes

B, T, D, H = 8, 1024, 512, 8
DH = D // H   # 64
HF = DH // 2  # 32
N_CORES = 8

_CACHE = {}


# ---------------------------------------------------------------- host prep

def _host_constants():
    f = np.arange(HF, dtype=np.float64)
    div = np.exp(f * -(math.log(10000.0) / (HF - 1)))
    j = np.arange(T, dtype=np.float64)[None, :]
    ang = div[:, None] * j                                   # [32, T]
    sin_j, cos_j = np.sin(ang), np.cos(ang)
    pe0T = np.concatenate([sin_j, cos_j], 0).astype(np.float16)     # [64, T]
    ctab = np.tile(cos_j, (4, 1))                                   # [128, T]
    stab = np.tile(np.concatenate([sin_j, -sin_j], 0), (2, 1))
    cs = np.concatenate([ctab, stab], 1).astype(np.float16)         # [128, 2T]
    return pe0T, np.ascontiguousarray(cs)


def _swap_cols(W):
    Wr = W.reshape(W.shape[0], H, 2, HF)
    return Wr[:, :, ::-1, :].reshape(W.shape[0], D)


def _swap_vec(v):
    return v.reshape(H, 2, HF)[:, ::-1, :].reshape(D)


# ---------------------------------------------------------------- bass build

def _build_nc():
    import concourse.bass as bass
    import concourse.mybir as mybir
    import concourse.tile as tile
    from concourse import bacc

    f32 = mybir.dt.float32
    f16 = mybir.dt.float16
    bf16 = mybir.dt.bfloat16

    nc = bacc.Bacc("TRN2")

    qT_d = nc.dram_tensor("qT", [D, T], f16, kind="ExternalInput")
    # wqaM: mp=0 columns of [Wq | swap(Wq)] packed kc-major; wqaR: mp=1..3
    wqaM_d = nc.dram_tensor("wqaM", [128, 1024], f16, kind="ExternalInput")
    wqaR_d = nc.dram_tensor("wqaR", [128, 3072], f16, kind="ExternalInput")
    kh01_d = nc.dram_tensor("kh01", [128, 2 * T], f16, kind="ExternalInput")
    kh27_d = nc.dram_tensor("kh27", [128, 6 * T], f16, kind="ExternalInput")
    vTwv_d = nc.dram_tensor("vTwv", [128, 6 * T], f16, kind="ExternalInput")
    wo2_d = nc.dram_tensor("wo2", [128, 2 * T], bf16, kind="ExternalInput")
    cs_d = nc.dram_tensor("cs", [128, 2 * T], f16, kind="ExternalInput")
    bqa_d = nc.dram_tensor("bqa", [128, 4], f32, kind="ExternalInput")
    vbqa_d = nc.dram_tensor("vbqa", [128, 8], f32, kind="ExternalInput")
    bvb_d = nc.dram_tensor("bvb", [D], f32, kind="ExternalInput")
    bob_d = nc.dram_tensor("bob", [D], f32, kind="ExternalInput")
    out_d = nc.dram_tensor("out", [T, D], f32, kind="ExternalOutput")

    AF = mybir.ActivationFunctionType
    ALU = mybir.AluOpType

    def bcast_ap(handle):
        base = handle[:]
        return bass.AP(tensor=base.tensor, offset=base.offset,
                       ap=[[0, 128]] + [list(x) for x in base.ap])

    with tile.TileContext(nc) as tc:
        with (
            tc.tile_pool(name="wpool", bufs=1) as wp,
            tc.tile_pool(name="tp", bufs=1) as tp,
            tc.tile_pool(name="ppm", bufs=1, space="PSUM") as ppm,
        ):
            # ---- persistent SBUF tiles
            cs_sb = wp.tile([128, 2 * T], f16, tag="cs")
            qT_sb = [wp.tile([128, T], f16, tag=f"qT{k}", name=f"qT{k}")
                     for k in range(4)]
            wqaM_sb = wp.tile([128, 1024], f16, tag="wqaM")
            wqaR_sb = wp.tile([128, 3072], f16, tag="wqaR")
            kh01_sb = wp.tile([128, 2 * T], f16, tag="kh01")
            kh27_sb = wp.tile([128, 6 * T], f16, tag="kh27")
            vTwv_sb = wp.tile([128, 6 * T], f16, tag="vTwv")
            wo2_sb = wp.tile([128, 2 * T], bf16, tag="wo2")
            bqa_sb = wp.tile([128, 4], f32, tag="bqa")
            vbqa_sb = wp.tile([128, 8], f32, tag="vbqa")
            bvb_sb = wp.tile([128, D], f32, tag="bvb")
            bob_sb = wp.tile([128, D], f32, tag="bob")
            QH = [wp.tile([128, T], f16, tag=f"QH{h}", name=f"QH{h}")
                  for h in range(8)]
            v_sb = [wp.tile([128, 8 * 65], bf16, tag=f"vsb{t}", name=f"vsb{t}")
                    for t in range(8)]
            pvT = [wp.tile([128, T], f16, tag=f"pvT{m}", name=f"pvT{m}")
                   for m in range(4)]

            def khat_ap(h, jc):
                if h < 2:
                    return kh01_sb[:, h * T + jc * 128:h * T + (jc + 1) * 128]
                return kh27_sb[:, (h - 2) * T + jc * 128:(h - 2) * T + (jc + 1) * 128]

            # ---- DMA issue plan.  One hardware engine drains each queue at
            # ~350 GB/s, so order IS the critical path.
            nc.sync.dma_start(out=cs_sb, in_=cs_d[:, :])
            nc.sync.dma_start(out=qT_sb[0], in_=qT_d[0:128, :])
            nc.sync.dma_start(out=qT_sb[1], in_=qT_d[128:256, :])
            nc.sync.dma_start(out=wqaM_sb, in_=wqaM_d[:, :])
            nc.sync.dma_start(out=qT_sb[2], in_=qT_d[256:384, :])
            nc.sync.dma_start(out=qT_sb[3], in_=qT_d[384:512, :])
            nc.sync.dma_start(out=wqaR_sb, in_=wqaR_d[:, :])
            nc.sync.dma_start(out=vTwv_sb, in_=vTwv_d[:, :])
            nc.sync.dma_start(out=kh01_sb, in_=kh01_d[:, :])
            nc.sync.dma_start(out=kh27_sb, in_=kh27_d[:, :])
            nc.sync.dma_start(out=wo2_sb, in_=wo2_d[:, :])
            # gpsimd software queue: only small loads
            nc.gpsimd.dma_start(out=bqa_sb, in_=bqa_d[:, :])
            nc.gpsimd.dma_start(out=vbqa_sb, in_=vbqa_d[:, :])
            nc.gpsimd.dma_start(out=bvb_sb, in_=bcast_ap(bvb_d))
            nc.gpsimd.dma_start(out=bob_sb, in_=bcast_ap(bob_d))
            # ones-columns of v_sb via memset (no DMA issue cost)
            for t in range(8):
                nc.gpsimd.memset(
                    v_sb[t].rearrange("p (h c) -> p h c", c=65)[:, :, 64:65], 1.0)

            ITERS = [(h, jc) for h in range(8) for jc in range(8)]
            pls = {}

            def emit_logits(k):
                h, jc = ITERS[k]
                pl = ppm.tile([128, T], f32, tag="pl", bufs=2, name="pl")
                for it in range(2):
                    nc.tensor.matmul(
                        pl[:, it * 512:(it + 1) * 512],
                        khat_ap(h, jc),
                        QH[h][:, it * 512:(it + 1) * 512],
                        start=True, stop=True)
                pls[k] = pl

            # ---- phase 1+2 PSUM in a scoped pool; the persistent pl ring
            # (ppm) lets head-0 logits start while phase 1 is still draining.
            ppA_cm = tc.tile_pool(name="ppA", bufs=1, space="PSUM")
            ppA = ppA_cm.__enter__()

            # warm-up matmuls: keep the PE busy through the HAM window while
            # input DMAs land, so real work runs at 2.4 GHz.
            warm = ppA.tile([128, 512], f32, tag="pq", bufs=2, name="warm")
            for w in range(10):
                nc.tensor.matmul(
                    warm[:, :], cs_sb[:, 0:128], cs_sb[:, 0:512],
                    start=(w == 0), stop=(w == 9))

            # ---- phase 1: q-aug projection + rotation -> QH tiles
            def emit_mp(mp):
                for n in range(2):
                    nsl = slice(n * 512, (n + 1) * 512)
                    pq = ppA.tile([128, 512], f32, tag="pq", bufs=2)
                    psw = ppA.tile([128, 512], f32, tag="psw", bufs=2)
                    for kc in range(4):
                        nc.tensor.matmul(
                            pq[:, :],
                            (wqaM_sb[:, kc * 128:(kc + 1) * 128] if mp == 0
                             else wqaR_sb[:, ((mp - 1) * 4 + kc) * 128:
                                          ((mp - 1) * 4 + kc + 1) * 128]),
                            qT_sb[kc][:, nsl],
                            start=(kc == 0), stop=(kc == 3))
                    for kc in range(4):
                        nc.tensor.matmul(
                            psw[:, :],
                            (wqaM_sb[:, 512 + kc * 128:512 + (kc + 1) * 128]
                             if mp == 0 else
                             wqaR_sb[:, 1536 + ((mp - 1) * 4 + kc) * 128:
                                     1536 + ((mp - 1) * 4 + kc + 1) * 128]),
                            qT_sb[kc][:, nsl],
                            start=(kc == 0), stop=(kc == 3))
                    # content halves (q + bq) on the scalar engine (idle
                    # until the exp chain starts)
                    nc.scalar.activation(
                        QH[2 * mp][0:64, nsl], pq[0:64, :],
                        AF.Identity, bias=bqa_sb[0:64, mp:mp + 1])
                    nc.scalar.activation(
                        QH[2 * mp + 1][0:64, nsl], pq[64:128, :],
                        AF.Identity, bias=bqa_sb[64:128, mp:mp + 1])
                    # rotation: t1=(pq+vbq)*C, t2=(psw+vbq_sw)*S, rot=t1+t2
                    t1 = tp.tile([128, 512], f16, tag="t1", bufs=2)
                    t2 = tp.tile([128, 512], f16, tag="t2", bufs=2)
                    nc.vector.scalar_tensor_tensor(
                        t1[:, :], pq[:, :], vbqa_sb[:, mp:mp + 1],
                        cs_sb[:, nsl], op0=ALU.add, op1=ALU.mult)
                    nc.vector.scalar_tensor_tensor(
                        t2[:, :], psw[:, :], vbqa_sb[:, mp + 4:mp + 5],
                        cs_sb[:, T + n * 512:T + (n + 1) * 512],
                        op0=ALU.add, op1=ALU.mult)
                    nc.vector.tensor_add(
                        QH[2 * mp][64:128, nsl], t1[0:64, :], t2[0:64, :])
                    nc.vector.tensor_add(
                        QH[2 * mp + 1][64:128, nsl], t1[64:128, :],
                        t2[64:128, :])

            emit_mp(0)
            emit_logits(0)
            emit_logits(1)

            # ---- phase 2: v projection, two token blocks per pl-ring tile
            for tpair in range(4):
                pv = ppm.tile([128, T], f32, tag="pl", bufs=2,
                              name=f"pv{tpair}")
                for half in range(2):
                    t = 2 * tpair + half
                    hsl = slice(half * 512, (half + 1) * 512)
                    for kc in range(4):
                        nc.tensor.matmul(
                            pv[:, hsl],
                            vTwv_sb[:, kc * T + t * 128:kc * T + (t + 1) * 128],
                            vTwv_sb[:, 4 * T + kc * 512:4 * T + (kc + 1) * 512],
                            start=(kc == 0), stop=(kc == 3))
                    nc.vector.tensor_add(
                        v_sb[t].rearrange("p (h c) -> p h c", c=65)[:, :, 0:64],
                        pv[:, hsl].rearrange("p (h c) -> p h c", c=64),
                        bvb_sb[:, :].rearrange("p (h c) -> p h c", c=64))

            for mp in range(1, 4):
                emit_mp(mp)
            ppA_cm.__exit__(None, None, None)

            # ---- phase 3: flat (h, jc) attention pipeline (ppv takes the
            # banks phase 1 just released)
            ppB_cm = tc.tile_pool(name="ppB", bufs=1, space="PSUM")
            ppB = ppB_cm.__enter__()
            ppv_cur = [None]
            for k, (h, jc) in enumerate(ITERS):
                pl = pls.pop(k)
                eT = tp.tile([128, T], bf16, tag="eT", bufs=8)
                nc.scalar.activation(eT[:, :], pl[:, :], AF.Exp)
                # logits lookahead BEFORE the PV matmuls: a stalled PV must
                # not head-of-line-block the exp chain.
                if k + 2 < len(ITERS):
                    emit_logits(k + 2)
                if jc == 0:
                    ppv_cur[0] = ppB.tile([65, T], f32, tag="ppv", bufs=2,
                                          name=f"ppv{h}")
                ppv = ppv_cur[0]
                for it in range(2):
                    nc.tensor.matmul(
                        ppv[:, it * 512:(it + 1) * 512],
                        v_sb[jc][:, h * 65:h * 65 + 65],
                        eT[:, it * 512:(it + 1) * 512],
                        start=(jc == 0), stop=(jc == 7))
                if jc == 7:
                    # normalization: r = 1/s (s = ones-row of ppv); the
                    # reciprocal bit-trick needs SBUF input, so copy first.
                    # Broadcast via the idle sync DMA queue, scale on vector.
                    scop = tp.tile([1, T], f32, tag="sc", bufs=2)
                    nc.vector.tensor_copy(scop[:, :], ppv[64:65, :])
                    r1 = tp.tile([1, T], f32, tag="r1", bufs=2)
                    nc.vector.reciprocal_approx_fast(r1[:, :], scop[:, :])
                    vflush = tp.tile([1, 4], f32, tag="vfl", bufs=2)
                    nc.vector.tensor_copy(vflush[:, :], bqa_sb[0:1, 0:4])
                    rbc = tp.tile([64, T], f32, tag="rbc", bufs=2)
                    r1b = r1[0:1, :]
                    nc.sync.dma_start(
                        out=rbc[:, :],
                        in_=bass.AP(tensor=r1b.tensor, offset=r1b.offset,
                                    ap=[list(r1b.ap[0]), [0, 64]]
                                    + [list(x) for x in r1b.ap[1:]]))
                    nc.vector.tensor_mul(
                        pvT[h // 2][(h % 2) * 64:(h % 2) * 64 + 64, :],
                        ppv[0:64, :], rbc[:, :])

            # tail warm-keepers: the last head's normalization hand-off would
            # otherwise idle the PE past the HAM window and phase 4 would run
            # at 1.2 GHz.  Discarded matmuls on a free ppv-ring tile.
            wkp = ppB.tile([65, T], f32, tag="ppv", bufs=2, name="wk")
            for w in range(10):
                nc.tensor.matmul(
                    wkp[:, 0:512], v_sb[0][:, 0:65], v_sb[0][:, 0:512],
                    start=(w == 0), stop=(w == 9))

            # ---- phase 4: output projection on the persistent pl ring (no
            # pool swap — overlaps the last head's normalization), two token
            # blocks per tile
            for tpair in range(4):
                po = ppm.tile([128, T], f32, tag="pl", bufs=2,
                              name=f"po{tpair}")
                for half in range(2):
                    t = 2 * tpair + half
                    hsl = slice(half * 512, (half + 1) * 512)
                    for kc in range(4):
                        nc.tensor.matmul(
                            po[:, hsl],
                            pvT[kc][:, t * 128:(t + 1) * 128],
                            wo2_sb[:, kc * 512:(kc + 1) * 512],
                            start=(kc == 0), stop=(kc == 3))
                    osb = tp.tile([128, 512], f32, tag="osb", bufs=3)
                    nc.vector.tensor_add(osb[:, :], po[:, hsl], bob_sb[:, :])
                    nc.sync.dma_start(out=out_d[t * 128:(t + 1) * 128, :],
                                      in_=osb[:, :])
            ppB_cm.__exit__(None, None, None)

    nc.finalize()
    return nc


def _get_nc():
    if "nc" not in _CACHE:
        _CACHE["nc"] = _build_nc()
    return _CACHE["nc"]


def _make_in_maps(query, key_in, value, Wq, bq, Wv, bv, Wo, bo, v_bias):
    pe0T, cs = _host_constants()
    Wq_aug = np.ascontiguousarray(
        np.concatenate([Wq, _swap_cols(Wq)], axis=1), dtype=np.float16)
    bq_aug = np.concatenate([bq, _swap_vec(bq)]).astype(np.float32)
    vb = v_bias.reshape(D).astype(np.float32)
    vbq_aug = (bq_aug + np.concatenate([vb, _swap_vec(vb)])).astype(np.float32)
    bqa = np.ascontiguousarray(bq_aug[:D].reshape(4, 128).T, dtype=np.float32)
    vbqa = np.ascontiguousarray(vbq_aug.reshape(8, 128).T, dtype=np.float32)

    # mp-packed q projections: wqaM = mp0 cols (kc-major, [Wq | swap] halves),
    # wqaR = mp1..3 cols.
    wqaM = np.empty((128, 1024), dtype=np.float16)
    wqaR = np.empty((128, 3072), dtype=np.float16)
    for kc in range(4):
        rows = slice(kc * 128, (kc + 1) * 128)
        wqaM[:, kc * 128:(kc + 1) * 128] = Wq_aug[rows, 0:128]
        wqaM[:, 512 + kc * 128:512 + (kc + 1) * 128] = Wq_aug[rows, D:D + 128]
        for mp in range(1, 4):
            c = ((mp - 1) * 4 + kc) * 128
            wqaR[:, c:c + 128] = Wq_aug[rows, mp * 128:(mp + 1) * 128]
            wqaR[:, 1536 + c:1536 + c + 128] = \
                Wq_aug[rows, D + mp * 128:D + (mp + 1) * 128]

    wv16 = Wv.astype(np.float16)
    wo2 = np.concatenate([Wo[kc * 128:(kc + 1) * 128, :] for kc in range(4)],
                         axis=1).astype(ml_dtypes.bfloat16)

    shared = {
        "wqaM": wqaM,
        "wqaR": wqaR,
        "wo2": np.ascontiguousarray(wo2),
        "cs": cs,
        "bqa": bqa,
        "vbqa": vbqa,
        "bvb": np.ascontiguousarray(bv, dtype=np.float32),
        "bob": np.ascontiguousarray(bo, dtype=np.float32),
    }
    in_maps = []
    for c in range(N_CORES):
        m = dict(shared)
        qT = query[c].T.astype(np.float16)
        kT = key_in[c].T.astype(np.float16)
        vT = value[c].T.astype(np.float16)
        m["qT"] = np.ascontiguousarray(qT)
        kh = np.empty((128, 8 * T), dtype=np.float16)
        for h in range(8):
            kh[0:64, h * T:(h + 1) * T] = kT[h * 64:(h + 1) * 64, :]
            kh[64:128, h * T:(h + 1) * T] = pe0T
        m["kh01"] = np.ascontiguousarray(kh[:, 0:2 * T])
        m["kh27"] = np.ascontiguousarray(kh[:, 2 * T:8 * T])
        vtwv = np.empty((128, 6 * T), dtype=np.float16)
        for kc in range(4):
            vtwv[:, kc * T:(kc + 1) * T] = vT[kc * 128:(kc + 1) * 128, :]
            vtwv[:, 4 * T + kc * 512:4 * T + (kc + 1) * 512] = \
                wv16[kc * 128:(kc + 1) * 128, :]
        m["vTwv"] = np.ascontiguousarray(vtwv)
        in_maps.append(m)
    return in_maps


def _run(in_maps, trace=False, tmpdir=None):
    from concourse.bass_utils import run_bass_kernel_spmd
    nc = _get_nc()
    return run_bass_kernel_spmd(nc, in_maps, core_ids=list(range(N_CORES)),
                                trace=trace, tmpdir=tmpdir)


def kernel(query, key_in, value, mask, Wq, bq, Wv, bv, Wo, bo, v_bias):
    query = np.asarray(query, dtype=np.float32)
    key_in = np.asarray(key_in, dtype=np.float32)
    value = np.asarray(value, dtype=np.float32)
    in_maps = _make_in_maps(query, key_in, value,
                            np.asarray(Wq, np.float32), np.asarray(bq, np.float32),
                            np.asarray(Wv, np.float32), np.asarray(bv, np.float32),
                            np.asarray(Wo, np.float32), np.asarray(bo, np.float32),
                            np.asarray(v_bias, np.float32))
    res = _run(in_maps, trace=False)
    out = np.stack([res.results[c]["out"] for c in range(N_CORES)], axis=0)
    return out.astype(np.float32)


def _install_ntff_shim():
    """The agent image's antenv lacks axon_hooks; provide it + register the
    ctypes NTFF hook from trn_agent_boot, and stub the artifact upload."""
    import types
    import antenv
    from concourse import bass_utils
    if "antenv.axon_hooks" not in sys.modules:
        mod = types.ModuleType("antenv.axon_hooks")
        mod._hook = None
        mod.set_axon_ntff_profile_hook = lambda h: setattr(mod, "_hook", h)
        mod.get_axon_ntff_profile_hook = lambda: mod._hook
        sys.modules["antenv.axon_hooks"] = mod
        antenv.axon_hooks = mod
        from trn_agent_boot.trn_boot import _ntff_profile_via_ctypes
        mod.set_axon_ntff_profile_hook(
            _ntff_profile_via_ctypes("/opt/axon/libaxon_pjrt.so"))
    bass_utils.upload_artifacts = lambda tmpdir: f"local:{tmpdir}"


def run_traced(query, key_in, value, mask, Wq, bq, Wv, bv, Wo, bo, v_bias,
               tmpdir=None):
    """Like kernel() but with NTFF profiling; returns (out, exec_time_ns)."""
    _install_ntff_shim()
    in_maps = _make_in_maps(
        np.asarray(query, np.float32), np.asarray(key_in, np.float32),
        np.asarray(value, np.float32),
        np.asarray(Wq, np.float32), np.asarray(bq, np.float32),
        np.asarray(Wv, np.float32), np.asarray(bv, np.float32),
        np.asarray(Wo, np.float32), np.asarray(bo, np.float32),
        np.asarray(v_bias, np.float32))
    res = _run(in_maps, trace=True, tmpdir=tmpdir)
    out = np.stack([res.results[c]["out"] for c in range(N_CORES)], axis=0)
    return out.astype(np.float32), res.exec_time_ns


# revision 52
# speedup vs baseline: 1.4510x; 1.4510x over previous
"""TENER multi-head self-attention Trainium2 kernel (8-core batch-parallel).

Math transformation (eliminates the [T,2T] skew/shift tensor entirely):
  rel[i,j] = (q_i + bq + v_bias_h) . pe(j-i),  pe(r)=[sin(r*d_f), cos(r*d_f)]
With angle-difference identities this becomes a plain matmul:
  rel[i,j] = a_i . sin(j*d) + b_i . cos(j*d)
so the full logits are one K=128 contraction of [q ; rot(q')] against
[k ; pe0] per head.  Softmax runs without max-subtraction (max logit ~61
< 88), row sums come from an appended ones-column in the PV matmul.

Matmul inputs are 16-bit: fp16 on the q/k/logits path (absolute logit
error ~5e-3, negligible through exp), bf16 for exp/value (exp reaches
e^61 and needs fp32-range exponents).

Structure: strictly sequential phases (projections, then attention,
then output) — each phase's PSUM pool closes before the next opens, and
within the attention phase a flat (head, key-block) pipeline keeps the
scalar engine's exp chain (the critical path) gapless with logits
emitted two iterations ahead.  Warm-up matmuls run during the initial
DMA window so the PE HAM clock gate reaches 2.4 GHz before real work.

Sharding: data-parallel over batch B=8 -> core c computes batch c.
"""
import math
import sys

sys.path.insert(0, "/opt/trn_rl_repo")

import numpy as np
import ml_dtypes

B, T, D, H = 8, 1024, 512, 8
DH = D // H   # 64
HF = DH // 2  # 32
N_CORES = 8

_CACHE = {}


# ---------------------------------------------------------------- host prep

def _host_constants():
    f = np.arange(HF, dtype=np.float64)
    div = np.exp(f * -(math.log(10000.0) / (HF - 1)))
    j = np.arange(T, dtype=np.float64)[None, :]
    ang = div[:, None] * j                                   # [32, T]
    sin_j, cos_j = np.sin(ang), np.cos(ang)
    pe0T = np.concatenate([sin_j, cos_j], 0).astype(np.float16)     # [64, T]
    ctab = np.tile(cos_j, (4, 1))                                   # [128, T]
    stab = np.tile(np.concatenate([sin_j, -sin_j], 0), (2, 1))
    cs = np.concatenate([ctab, stab], 1).astype(np.float16)         # [128, 2T]
    return pe0T, np.ascontiguousarray(cs)


def _swap_cols(W):
    Wr = W.reshape(W.shape[0], H, 2, HF)
    return Wr[:, :, ::-1, :].reshape(W.shape[0], D)


def _swap_vec(v):
    return v.reshape(H, 2, HF)[:, ::-1, :].reshape(D)


# ---------------------------------------------------------------- bass build

def _build_nc():
    import concourse.bass as bass
    import concourse.mybir as mybir
    import concourse.tile as tile
    from concourse import bacc

    f32 = mybir.dt.float32
    f16 = mybir.dt.float16
    bf16 = mybir.dt.bfloat16

    nc = bacc.Bacc("TRN2")

    qT_d = nc.dram_tensor("qT", [D, T], f16, kind="ExternalInput")
    # wqaM: mp=0 columns of [Wq | swap(Wq)] packed kc-major; wqaR: mp=1..3
    wqaM_d = nc.dram_tensor("wqaM", [128, 1024], f16, kind="ExternalInput")
    wqaR_d = nc.dram_tensor("wqaR", [128, 3072], f16, kind="ExternalInput")
    kh01_d = nc.dram_tensor("kh01", [128, 2 * T], f16, kind="ExternalInput")
    kh27_d = nc.dram_tensor("kh27", [128, 6 * T], f16, kind="ExternalInput")
    vTwv_d = nc.dram_tensor("vTwv", [128, 6 * T], f16, kind="ExternalInput")
    wo2_d = nc.dram_tensor("wo2", [128, 2 * T], bf16, kind="ExternalInput")
    cs_d = nc.dram_tensor("cs", [128, 2 * T], f16, kind="ExternalInput")
    bqa_d = nc.dram_tensor("bqa", [128, 4], f32, kind="ExternalInput")
    vbqa_d = nc.dram_tensor("vbqa", [128, 8], f32, kind="ExternalInput")
    bvb_d = nc.dram_tensor("bvb", [D], f32, kind="ExternalInput")
    bob_d = nc.dram_tensor("bob", [D], f32, kind="ExternalInput")
    out_d = nc.dram_tensor("out", [T, D], f32, kind="ExternalOutput")

    AF = mybir.ActivationFunctionType
    ALU = mybir.AluOpType

    def bcast_ap(handle):
        base = handle[:]
        return bass.AP(tensor=base.tensor, offset=base.offset,
                       ap=[[0, 128]] + [list(x) for x in base.ap])

    with tile.TileContext(nc) as tc:
        with (
            tc.tile_pool(name="wpool", bufs=1) as wp,
            tc.tile_pool(name="tp", bufs=1) as tp,
            tc.tile_pool(name="ppm", bufs=1, space="PSUM") as ppm,
        ):
            # ---- persistent SBUF tiles
            cs_sb = wp.tile([128, 2 * T], f16, tag="cs")
            qT_sb = [wp.tile([128, T], f16, tag=f"qT{k}", name=f"qT{k}")
                     for k in range(4)]
            wqaM_sb = wp.tile([128, 1024], f16, tag="wqaM")
            wqaR_sb = wp.tile([128, 3072], f16, tag="wqaR")
            kh01_sb = wp.tile([128, 2 * T], f16, tag="kh01")
            kh27_sb = wp.tile([128, 6 * T], f16, tag="kh27")
            vTwv_sb = wp.tile([128, 6 * T], f16, tag="vTwv")
            wo2_sb = wp.tile([128, 2 * T], bf16, tag="wo2")
            bqa_sb = wp.tile([128, 4], f32, tag="bqa")
            vbqa_sb = wp.tile([128, 8], f32, tag="vbqa")
            bvb_sb = wp.tile([128, D], f32, tag="bvb")
            bob_sb = wp.tile([128, D], f32, tag="bob")
            QH = [wp.tile([128, T], f16, tag=f"QH{h}", name=f"QH{h}")
                  for h in range(8)]
            v_sb = [wp.tile([128, 8 * 65], bf16, tag=f"vsb{t}", name=f"vsb{t}")
                    for t in range(8)]
            pvT = [wp.tile([128, T], f16, tag=f"pvT{m}", name=f"pvT{m}")
                   for m in range(4)]

            def khat_ap(h, jc):
                if h < 2:
                    return kh01_sb[:, h * T + jc * 128:h * T + (jc + 1) * 128]
                return kh27_sb[:, (h - 2) * T + jc * 128:(h - 2) * T + (jc + 1) * 128]

            # ---- DMA issue plan.  One hardware engine drains each queue at
            # ~350 GB/s, so order IS the critical path.
            nc.sync.dma_start(out=cs_sb, in_=cs_d[:, :])
            nc.sync.dma_start(out=qT_sb[0], in_=qT_d[0:128, :])
            nc.sync.dma_start(out=qT_sb[1], in_=qT_d[128:256, :])
            nc.sync.dma_start(out=wqaM_sb, in_=wqaM_d[:, :])
            nc.sync.dma_start(out=qT_sb[2], in_=qT_d[256:384, :])
            nc.sync.dma_start(out=qT_sb[3], in_=qT_d[384:512, :])
            nc.sync.dma_start(out=wqaR_sb, in_=wqaR_d[:, :])
            nc.sync.dma_start(out=vTwv_sb, in_=vTwv_d[:, :])
            nc.sync.dma_start(out=kh01_sb, in_=kh01_d[:, :])
            nc.sync.dma_start(out=kh27_sb, in_=kh27_d[:, :])
            nc.sync.dma_start(out=wo2_sb, in_=wo2_d[:, :])
            # gpsimd software queue: only small loads
            nc.gpsimd.dma_start(out=bqa_sb, in_=bqa_d[:, :])
            nc.gpsimd.dma_start(out=vbqa_sb, in_=vbqa_d[:, :])
            nc.gpsimd.dma_start(out=bvb_sb, in_=bcast_ap(bvb_d))
            nc.gpsimd.dma_start(out=bob_sb, in_=bcast_ap(bob_d))
            # ones-columns of v_sb via memset (no DMA issue cost)
            for t in range(8):
                nc.gpsimd.memset(
                    v_sb[t].rearrange("p (h c) -> p h c", c=65)[:, :, 64:65], 1.0)

            ITERS = [(h, jc) for h in range(8) for jc in range(8)]
            pls = {}

            def emit_logits(k):
                h, jc = ITERS[k]
                pl = ppm.tile([128, T], f32, tag="pl", bufs=2, name="pl")
                for it in range(2):
                    nc.tensor.matmul(
                        pl[:, it * 512:(it + 1) * 512],
                        khat_ap(h, jc),
                        QH[h][:, it * 512:(it + 1) * 512],
                        start=True, stop=True)
                pls[k] = pl

            # ---- phase 1+2 PSUM in a scoped pool; the persistent pl ring
            # (ppm) lets head-0 logits start while phase 1 is still draining.
            ppA_cm = tc.tile_pool(name="ppA", bufs=1, space="PSUM")
            ppA = ppA_cm.__enter__()

            # warm-up matmuls: keep the PE busy through the HAM window while
            # input DMAs land, so real work runs at 2.4 GHz.
            warm = ppA.tile([128, 512], f32, tag="pq", bufs=2, name="warm")
            for w in range(10):
                nc.tensor.matmul(
                    warm[:, :], cs_sb[:, 0:128], cs_sb[:, 0:512],
                    start=(w == 0), stop=(w == 9))

            # ---- phase 1: q-aug projection + rotation -> QH tiles
            def emit_mp(mp):
                for n in range(2):
                    nsl = slice(n * 512, (n + 1) * 512)
                    pq = ppA.tile([128, 512], f32, tag="pq", bufs=2)
                    psw = ppA.tile([128, 512], f32, tag="psw", bufs=2)
                    for kc in range(4):
                        nc.tensor.matmul(
                            pq[:, :],
                            (wqaM_sb[:, kc * 128:(kc + 1) * 128] if mp == 0
                             else wqaR_sb[:, ((mp - 1) * 4 + kc) * 128:
                                          ((mp - 1) * 4 + kc + 1) * 128]),
                            qT_sb[kc][:, nsl],
                            start=(kc == 0), stop=(kc == 3))
                    for kc in range(4):
                        nc.tensor.matmul(
                            psw[:, :],
                            (wqaM_sb[:, 512 + kc * 128:512 + (kc + 1) * 128]
                             if mp == 0 else
                             wqaR_sb[:, 1536 + ((mp - 1) * 4 + kc) * 128:
                                     1536 + ((mp - 1) * 4 + kc + 1) * 128]),
                            qT_sb[kc][:, nsl],
                            start=(kc == 0), stop=(kc == 3))
                    # content halves (q + bq) on the scalar engine (idle
                    # until the exp chain starts)
                    nc.scalar.activation(
                        QH[2 * mp][0:64, nsl], pq[0:64, :],
                        AF.Identity, bias=bqa_sb[0:64, mp:mp + 1])
                    nc.scalar.activation(
                        QH[2 * mp + 1][0:64, nsl], pq[64:128, :],
                        AF.Identity, bias=bqa_sb[64:128, mp:mp + 1])
                    # rotation: t1=(pq+vbq)*C, t2=(psw+vbq_sw)*S, rot=t1+t2
                    t1 = tp.tile([128, 512], f16, tag="t1", bufs=2)
                    t2 = tp.tile([128, 512], f16, tag="t2", bufs=2)
                    nc.vector.scalar_tensor_tensor(
                        t1[:, :], pq[:, :], vbqa_sb[:, mp:mp + 1],
                        cs_sb[:, nsl], op0=ALU.add, op1=ALU.mult)
                    nc.vector.scalar_tensor_tensor(
                        t2[:, :], psw[:, :], vbqa_sb[:, mp + 4:mp + 5],
                        cs_sb[:, T + n * 512:T + (n + 1) * 512],
                        op0=ALU.add, op1=ALU.mult)
                    nc.vector.tensor_add(
                        QH[2 * mp][64:128, nsl], t1[0:64, :], t2[0:64, :])
                    nc.vector.tensor_add(
                        QH[2 * mp + 1][64:128, nsl], t1[64:128, :],
                        t2[64:128, :])

            emit_mp(0)
            emit_logits(0)
            emit_logits(1)

            # ---- phase 2: v projection, two token blocks per pl-ring tile
            for tpair in range(4):
                pv = ppm.tile([128, T], f32, tag="pl", bufs=2,
                              name=f"pv{tpair}")
                for half in range(2):
                    t = 2 * tpair + half
                    hsl = slice(half * 512, (half + 1) * 512)
                    for kc in range(4):
                        nc.tensor.matmul(
                            pv[:, hsl],
                            vTwv_sb[:, kc * T + t * 128:kc * T + (t + 1) * 128],
                            vTwv_sb[:, 4 * T + kc * 512:4 * T + (kc + 1) * 512],
                            start=(kc == 0), stop=(kc == 3))
                    nc.vector.tensor_add(
                        v_sb[t].rearrange("p (h c) -> p h c", c=65)[:, :, 0:64],
                        pv[:, hsl].rearrange("p (h c) -> p h c", c=64),
                        bvb_sb[:, :].rearrange("p (h c) -> p h c", c=64))

            for mp in range(1, 4):
                emit_mp(mp)
            ppA_cm.__exit__(None, None, None)

            # ---- phase 3: flat (h, jc) attention pipeline (ppv takes the
            # banks phase 1 just released)
            ppB_cm = tc.tile_pool(name="ppB", bufs=1, space="PSUM")
            ppB = ppB_cm.__enter__()
            ppv_cur = [None]
            for k, (h, jc) in enumerate(ITERS):
                pl = pls.pop(k)
                eT = tp.tile([128, T], bf16, tag="eT", bufs=8)
                nc.scalar.activation(eT[:, :], pl[:, :], AF.Exp)
                # logits lookahead BEFORE the PV matmuls: a stalled PV must
                # not head-of-line-block the exp chain.
                if k + 2 < len(ITERS):
                    emit_logits(k + 2)
                if jc == 0:
                    ppv_cur[0] = ppB.tile([65, T], f32, tag="ppv", bufs=2,
                                          name=f"ppv{h}")
                ppv = ppv_cur[0]
                for it in range(2):
                    nc.tensor.matmul(
                        ppv[:, it * 512:(it + 1) * 512],
                        v_sb[jc][:, h * 65:h * 65 + 65],
                        eT[:, it * 512:(it + 1) * 512],
                        start=(jc == 0), stop=(jc == 7))
                if jc == 7:
                    # normalization: r = 1/s (s = ones-row of ppv); the
                    # reciprocal bit-trick needs SBUF input, so copy first.
                    # Broadcast via the idle sync DMA queue, scale on vector.
                    scop = tp.tile([1, T], f32, tag="sc", bufs=2)
                    nc.vector.tensor_copy(scop[:, :], ppv[64:65, :])
                    r1 = tp.tile([1, T], f32, tag="r1", bufs=2)
                    nc.vector.reciprocal_approx_fast(r1[:, :], scop[:, :])
                    vflush = tp.tile([1, 4], f32, tag="vfl", bufs=2)
                    nc.vector.tensor_copy(vflush[:, :], bqa_sb[0:1, 0:4])
                    rbc = tp.tile([128, T], f32, tag="rbc", bufs=2)
                    nc.gpsimd.partition_broadcast(rbc[:, :], r1[:, :])
                    gflush = tp.tile([1, 4], f32, tag="gfl", bufs=2)
                    nc.gpsimd.memset(gflush[:, :], 0.0)
                    nc.vector.tensor_mul(
                        pvT[h // 2][(h % 2) * 64:(h % 2) * 64 + 64, :],
                        ppv[0:64, :], rbc[0:64, :])

            # tail warm-keepers: the last head's normalization hand-off would
            # otherwise idle the PE past the HAM window and phase 4 would run
            # at 1.2 GHz.  Discarded matmuls on a free ppv-ring tile.
            wkp = ppB.tile([65, T], f32, tag="ppv", bufs=2, name="wk")
            for w in range(10):
                nc.tensor.matmul(
                    wkp[:, 0:512], v_sb[0][:, 0:65], v_sb[0][:, 0:512],
                    start=(w == 0), stop=(w == 9))

            # ---- phase 4: output projection on the persistent pl ring (no
            # pool swap — overlaps the last head's normalization), two token
            # blocks per tile
            for tpair in range(4):
                po = ppm.tile([128, T], f32, tag="pl", bufs=2,
                              name=f"po{tpair}")
                for half in range(2):
                    t = 2 * tpair + half
                    hsl = slice(half * 512, (half + 1) * 512)
                    for kc in range(4):
                        nc.tensor.matmul(
                            po[:, hsl],
                            pvT[kc][:, t * 128:(t + 1) * 128],
                            wo2_sb[:, kc * 512:(kc + 1) * 512],
                            start=(kc == 0), stop=(kc == 3))
                    osb = tp.tile([128, 512], f32, tag="osb", bufs=3)
                    nc.vector.tensor_add(osb[:, :], po[:, hsl], bob_sb[:, :])
                    nc.sync.dma_start(out=out_d[t * 128:(t + 1) * 128, :],
                                      in_=osb[:, :])
            ppB_cm.__exit__(None, None, None)

    nc.finalize()
    return nc


def _get_nc():
    if "nc" not in _CACHE:
        _CACHE["nc"] = _build_nc()
    return _CACHE["nc"]


def _make_in_maps(query, key_in, value, Wq, bq, Wv, bv, Wo, bo, v_bias):
    pe0T, cs = _host_constants()
    Wq_aug = np.ascontiguousarray(
        np.concatenate([Wq, _swap_cols(Wq)], axis=1), dtype=np.float16)
    bq_aug = np.concatenate([bq, _swap_vec(bq)]).astype(np.float32)
    vb = v_bias.reshape(D).astype(np.float32)
    vbq_aug = (bq_aug + np.concatenate([vb, _swap_vec(vb)])).astype(np.float32)
    bqa = np.ascontiguousarray(bq_aug[:D].reshape(4, 128).T, dtype=np.float32)
    vbqa = np.ascontiguousarray(vbq_aug.reshape(8, 128).T, dtype=np.float32)

    # mp-packed q projections: wqaM = mp0 cols (kc-major, [Wq | swap] halves),
    # wqaR = mp1..3 cols.
    wqaM = np.empty((128, 1024), dtype=np.float16)
    wqaR = np.empty((128, 3072), dtype=np.float16)
    for kc in range(4):
        rows = slice(kc * 128, (kc + 1) * 128)
        wqaM[:, kc * 128:(kc + 1) * 128] = Wq_aug[rows, 0:128]
        wqaM[:, 512 + kc * 128:512 + (kc + 1) * 128] = Wq_aug[rows, D:D + 128]
        for mp in range(1, 4):
            c = ((mp - 1) * 4 + kc) * 128
            wqaR[:, c:c + 128] = Wq_aug[rows, mp * 128:(mp + 1) * 128]
            wqaR[:, 1536 + c:1536 + c + 128] = \
                Wq_aug[rows, D + mp * 128:D + (mp + 1) * 128]

    wv16 = Wv.astype(np.float16)
    wo2 = np.concatenate([Wo[kc * 128:(kc + 1) * 128, :] for kc in range(4)],
                         axis=1).astype(ml_dtypes.bfloat16)

    shared = {
        "wqaM": wqaM,
        "wqaR": wqaR,
        "wo2": np.ascontiguousarray(wo2),
        "cs": cs,
        "bqa": bqa,
        "vbqa": vbqa,
        "bvb": np.ascontiguousarray(bv, dtype=np.float32),
        "bob": np.ascontiguousarray(bo, dtype=np.float32),
    }
    in_maps = []
    for c in range(N_CORES):
        m = dict(shared)
        qT = query[c].T.astype(np.float16)
        kT = key_in[c].T.astype(np.float16)
        vT = value[c].T.astype(np.float16)
        m["qT"] = np.ascontiguousarray(qT)
        kh = np.empty((128, 8 * T), dtype=np.float16)
        for h in range(8):
            kh[0:64, h * T:(h + 1) * T] = kT[h * 64:(h + 1) * 64, :]
            kh[64:128, h * T:(h + 1) * T] = pe0T
        m["kh01"] = np.ascontiguousarray(kh[:, 0:2 * T])
        m["kh27"] = np.ascontiguousarray(kh[:, 2 * T:8 * T])
        vtwv = np.empty((128, 6 * T), dtype=np.float16)
        for kc in range(4):
            vtwv[:, kc * T:(kc + 1) * T] = vT[kc * 128:(kc + 1) * 128, :]
            vtwv[:, 4 * T + kc * 512:4 * T + (kc + 1) * 512] = \
                wv16[kc * 128:(kc + 1) * 128, :]
        m["vTwv"] = np.ascontiguousarray(vtwv)
        in_maps.append(m)
    return in_maps


def _run(in_maps, trace=False, tmpdir=None):
    from concourse.bass_utils import run_bass_kernel_spmd
    nc = _get_nc()
    return run_bass_kernel_spmd(nc, in_maps, core_ids=list(range(N_CORES)),
                                trace=trace, tmpdir=tmpdir)


def kernel(query, key_in, value, mask, Wq, bq, Wv, bv, Wo, bo, v_bias):
    query = np.asarray(query, dtype=np.float32)
    key_in = np.asarray(key_in, dtype=np.float32)
    value = np.asarray(value, dtype=np.float32)
    in_maps = _make_in_maps(query, key_in, value,
                            np.asarray(Wq, np.float32), np.asarray(bq, np.float32),
                            np.asarray(Wv, np.float32), np.asarray(bv, np.float32),
                            np.asarray(Wo, np.float32), np.asarray(bo, np.float32),
                            np.asarray(v_bias, np.float32))
    res = _run(in_maps, trace=False)
    out = np.stack([res.results[c]["out"] for c in range(N_CORES)], axis=0)
    return out.astype(np.float32)


def _install_ntff_shim():
    """The agent image's antenv lacks axon_hooks; provide it + register the
    ctypes NTFF hook from trn_agent_boot, and stub the artifact upload."""
    import types
    import antenv
    from concourse import bass_utils
    if "antenv.axon_hooks" not in sys.modules:
        mod = types.ModuleType("antenv.axon_hooks")
        mod._hook = None
        mod.set_axon_ntff_profile_hook = lambda h: setattr(mod, "_hook", h)
        mod.get_axon_ntff_profile_hook = lambda: mod._hook
        sys.modules["antenv.axon_hooks"] = mod
        antenv.axon_hooks = mod
        from trn_agent_boot.trn_boot import _ntff_profile_via_ctypes
        mod.set_axon_ntff_profile_hook(
            _ntff_profile_via_ctypes("/opt/axon/libaxon_pjrt.so"))
    bass_utils.upload_artifacts = lambda tmpdir: f"local:{tmpdir}"


def run_traced(query, key_in, value, mask, Wq, bq, Wv, bv, Wo, bo, v_bias,
               tmpdir=None):
    """Like kernel() but with NTFF profiling; returns (out, exec_time_ns)."""
    _install_ntff_shim()
    in_maps = _make_in_maps(
        np.asarray(query, np.float32), np.asarray(key_in, np.float32),
        np.asarray(value, np.float32),
        np.asarray(Wq, np.float32), np.asarray(bq, np.float32),
        np.asarray(Wv, np.float32), np.asarray(bv, np.float32),
        np.asarray(Wo, np.float32), np.asarray(bo, np.float32),
        np.asarray(v_bias, np.float32))
    res = _run(in_maps, trace=True, tmpdir=tmpdir)
    out = np.stack([res.results[c]["out"] for c in range(N_CORES)], axis=0)
    return out.astype(np.float32), res.exec_time_ns
